# revision 21
# baseline (speedup 1.0000x reference)
"""CropAndResize (TF crop_and_resize, bilinear, extrapolation=0) on 8 Trainium2 cores.

Problem (hardcoded): image [8, 256, 200, 200] f32, boxes [512, 4] f32 (y1,x1,y2,x2
normalized), box_ind [512] int32, crop 14x14 -> out [512, 256, 14, 14] f32.

Sharding: boxes split 64/core across 8 cores; image replicated (in HBM) per core.

Strategy (SPMD-safe: one program; all per-core variation flows through input tables):
  * Host re-lays the image out as [B, H, W, C] (channels innermost) so that, for any
    sampling point, the 2x256 values (x-pair x channels) are one contiguous 512-element
    run in HBM.
  * Host computes every box's bilinear sample coordinates and weights; validity masks
    are folded into the weights (so out-of-range taps contribute 0 = extrapolation).
  * Device: per q-chunk, one indirect DMA gathers [128, CQ, 2k, 2m*256c] with a
    host-built int32 offset table (one offset per (slot, k); block = 512 elements,
    512B/1KB aligned). Slots are (n, i, j) output cells: p = 2n + (ij>=98),
    q = ij % 98. Then 7 tensor_tensor ops apply out = sum_km X_km * W_km with
    per-(k,m) combined weight tiles broadcast along channels. Output is written as
    [slot, 256] (channels innermost); host restores [512, 256, 14, 14].
"""

import numpy as np

import concourse.bass as bass
import concourse.mybir as mybir
from concourse.tile import TileContext
from concourse import bass_utils

B, C, H, W = 8, 256, 200, 200
N_BOXES = 512
CH, CW = 14, 14
N_CORES = 8
NB = N_BOXES // N_CORES  # 64 boxes per core
IJ = CH * CW  # 196
QCOLS = IJ // 2  # 98 q-columns (p = 2n + ij//98)
CQ = 7  # q-columns per chunk
N_CHUNKS = QCOLS // CQ  # 14
IMG_ELEMS = B * H * W * C
IMG_PAD = 1024

F32 = mybir.dt.float32
I32 = mybir.dt.int32

_NC_CACHE = {}
TRACE_MODE = False  # set True (e.g. from test.py) to collect an NTFF profile
LAST_RESULT = {}  # exec_time_ns etc. from the most recent run


def _host_prep(boxes, box_ind):
    """Per-core offset tables and combined weight tiles."""
    boxes = np.asarray(boxes, np.float32)
    box_ind = np.asarray(box_ind, np.int64)

    y1, x1, y2, x2 = boxes[:, 0], boxes[:, 1], boxes[:, 2], boxes[:, 3]
    one = np.float32(1.0)

    def grid(a, b2, size, dim):
        scale = (b2 - a) * np.float32(dim - 1) / np.float32(size - 1)
        pos = a[:, None] * np.float32(dim - 1) + np.arange(size, dtype=np.float32) * scale[:, None]
        valid = (pos >= 0) & (pos <= dim - 1)
        pc = np.clip(pos, 0, dim - 1)
        rr = np.minimum(np.floor(pc), dim - 2).astype(np.int64)
        w_hi = pc - rr.astype(np.float32)  # weight of tap rr+1
        w_lo = one - w_hi
        vf = valid.astype(np.float32)
        return rr, w_lo * vf, w_hi * vf

    r, wy0, wy1 = grid(y1, y2, CH, H)  # [512,14]
    c, wx0, wx1 = grid(x1, x2, CW, W)  # [512,14]

    idx_maps = []
    w_maps = []
    for g in range(N_CORES):
        sl = slice(g * NB, (g + 1) * NB)
        bg, rg, cg = box_ind[sl], r[sl], c[sl]
        wy0g, wy1g, wx0g, wx1g = wy0[sl], wy1[sl], wx0[sl], wx1[sl]

        # slot (n, ij) -> p = 2n + ij//98, q = ij%98; table col = q*2 + k
        n_of_p = np.arange(128) // 2  # [128]
        h_of_p = np.arange(128) % 2
        ij_pq = h_of_p[:, None] * QCOLS + np.arange(QCOLS)[None, :]  # [128, 98]
        i_pq = ij_pq // CW
        j_pq = ij_pq % CW
        n_pq = np.broadcast_to(n_of_p[:, None], (128, QCOLS))

        b_pq = bg[n_pq]  # [128, 98]
        r_pq = rg[n_pq, i_pq]
        c_pq = cg[n_pq, j_pq]
        # offset for (p, q, k): ((b*H + r + k) * W + c) * C
        idx = ((b_pq[:, :, None] * H + r_pq[:, :, None] + np.arange(2)[None, None, :]) * W
               + c_pq[:, :, None]) * C  # [128, 98, 2]
        idx_maps.append(np.ascontiguousarray(idx.reshape(128, QCOLS * 2).astype(np.int32)))

        # combined weights per (k, m): wy_k * wx_m -> [4, 128, 98], (k,m) flattened k-major
        wy = np.stack([wy0g[n_pq, i_pq], wy1g[n_pq, i_pq]])  # [2,128,98]
        wx = np.stack([wx0g[n_pq, j_pq], wx1g[n_pq, j_pq]])  # [2,128,98]
        wkm = (wy[:, None] * wx[None, :]).reshape(4, 128, QCOLS)
        # [128, 4*QCOLS] host-side transpose so the device DMA is a plain 2-D copy
        w_maps.append(
            np.ascontiguousarray(wkm.transpose(1, 0, 2).reshape(128, 4 * QCOLS).astype(np.float32))
        )

    return idx_maps, w_maps


def _legalize_waits(nc, max_waits=1):
    """This toolchain's codegen rejects instructions with more than one sync
    wait. Hoist extra waits onto same-engine NoOps inserted right before the
    offending instruction (semantically identical: the engine stream blocks
    on each wait in turn)."""
    nop_id = 0
    for blk in nc.m.functions[0].blocks:
        new_list = []
        for inst in blk.instructions:
            si = inst.sync_info
            waits = list(si.on_wait) if si is not None else []
            if len(waits) > max_waits:
                extra, keep = waits[:-max_waits], waits[-max_waits:]
                import bass_rust as _br

                for w in extra:
                    nop = mybir.InstNoOp(name=f"waitnop_{nop_id}", ins=[], outs=[])
                    nop_id += 1
                    nop.engine = inst.engine
                    nop.sync_info = _br.SyncInfo(on_wait=[w], on_update=[])
                    nc.inst_map[nop.name] = nop
                    new_list.append(nop)
                si.on_wait = keep
            new_list.append(inst)
        blk.instructions = new_list


def _build_nc():
    nc = bass.Bass()
    img = nc.dram_tensor("img", [1, IMG_ELEMS + IMG_PAD], F32, kind="ExternalInput")
    idxt = nc.dram_tensor("idxt", [128, QCOLS * 2], I32, kind="ExternalInput")
    wt = nc.dram_tensor("wt", [128, 4 * QCOLS], F32, kind="ExternalInput")
    out = nc.dram_tensor("out", [128, QCOLS, C], F32, kind="ExternalOutput")

    with TileContext(nc) as tc:
        with (
            tc.tile_pool(name="const", bufs=1) as cpool,
            tc.tile_pool(name="gat", bufs=3) as gpool,
            tc.tile_pool(name="work", bufs=2) as wpool,
            tc.tile_pool(name="outp", bufs=2) as opool,
        ):
            idx_s = cpool.tile([128, QCOLS * 2], I32)
            nc.sync.dma_start(out=idx_s[:], in_=idxt[:])
            w_s = cpool.tile([128, 4 * QCOLS], F32)
            nc.sync.dma_start(out=w_s[:], in_=wt[:])
            tc.strict_bb_all_engine_barrier()

            mult, add = mybir.AluOpType.mult, mybir.AluOpType.add
            for ch in range(N_CHUNKS):
                q0 = ch * CQ
                gt = gpool.tile([128, CQ * 2 * 2 * C], F32, tag="g")
                gv = gt[:].rearrange("p (q k m c) -> p q k m c", q=CQ, k=2, m=2)
                # HW indirect DMA consumes ONE offset per partition (block =
                # out free size), so issue one call per (q, k) table column.
                for qq in range(CQ):
                    for k in range(2):
                        col = (q0 + qq) * 2 + k
                        nc.gpsimd.indirect_dma_start(
                            out=gt[:, (qq * 4 + k * 2) * C : (qq * 4 + k * 2 + 2) * C],
                            out_offset=None,
                            in_=img[:],
                            in_offset=bass.IndirectOffsetOnAxis(
                                ap=idx_s[:, col : col + 1], axis=1
                            ),
                        )

                def wb(k, m, q0=q0):
                    # combined weight [128, CQ] -> broadcast [128, CQ, C]
                    colw = (2 * k + m) * QCOLS + q0
                    return (
                        w_s[:, colw : colw + CQ]
                        .unsqueeze(2)
                        .broadcast_to([128, CQ, C])
                    )

                t0 = wpool.tile([128, CQ, C], F32, tag="t0")
                t1 = wpool.tile([128, CQ, C], F32, tag="t1")
                ot = opool.tile([128, CQ, C], F32, tag="ot")
                nc.vector.tensor_tensor(out=t0[:], in0=gv[:, :, 0, 0, :], in1=wb(0, 0), op=mult)
                nc.vector.tensor_tensor(out=t1[:], in0=gv[:, :, 0, 1, :], in1=wb(0, 1), op=mult)
                nc.vector.tensor_tensor(out=t0[:], in0=t0[:], in1=t1[:], op=add)
                nc.gpsimd.tensor_tensor(out=t1[:], in0=gv[:, :, 1, 0, :], in1=wb(1, 0), op=mult)
                nc.gpsimd.tensor_tensor(out=ot[:], in0=gv[:, :, 1, 1, :], in1=wb(1, 1), op=mult)
                nc.gpsimd.tensor_tensor(out=t1[:], in0=t1[:], in1=ot[:], op=add)
                nc.vector.tensor_tensor(out=ot[:], in0=t0[:], in1=t1[:], op=add)
                nc.sync.dma_start(out=out[:, q0 : q0 + CQ, :], in_=ot[:])

    _legalize_waits(nc)
    return nc


def kernel(image, boxes, box_ind, crop_height, crop_width):
    assert int(crop_height) == CH and int(crop_width) == CW
    image = np.asarray(image, dtype=np.float32)
    assert image.shape == (B, C, H, W)

    idx_maps, w_maps = _host_prep(boxes, box_ind)

    img_flat = np.zeros((1, IMG_ELEMS + IMG_PAD), np.float32)
    img_flat[0, :IMG_ELEMS] = image.transpose(0, 2, 3, 1).reshape(-1)  # [B,H,W,C]

    in_maps = [
        {"img": img_flat, "idxt": idx_maps[g], "wt": w_maps[g]} for g in range(N_CORES)
    ]

    if "nc" not in _NC_CACHE:
        _NC_CACHE["nc"] = _build_nc()
    nc = _NC_CACHE["nc"]
    try:
        res = bass_utils.run_bass_kernel_spmd(
            nc, in_maps, core_ids=list(range(N_CORES)), trace=TRACE_MODE
        )
    except ModuleNotFoundError:
        # NTFF profiling hook unavailable in this environment
        res = bass_utils.run_bass_kernel_spmd(
            nc, in_maps, core_ids=list(range(N_CORES)), trace=False
        )
    LAST_RESULT["exec_time_ns"] = res.exec_time_ns
    LAST_RESULT["mean_exec_time_ns"] = res.mean_exec_time_ns
    LAST_RESULT["profile_json"] = res.profile_json
    LAST_RESULT["insts_and_trace"] = res.instructions_and_trace

    out = np.empty((N_BOXES, C, CH, CW), np.float32)
    for g in range(N_CORES):
        o = np.asarray(res.results[g]["out"])  # [128, 98, 256]
        o = o.reshape(NB, 2, QCOLS, C).reshape(NB, IJ, C)  # [n, ij, c]
        out[g * NB : (g + 1) * NB] = o.transpose(0, 2, 1).reshape(NB, C, CH, CW)
    return out
